# revision 13
# baseline (speedup 1.0000x reference)
"""Trainium2 Bass kernel for the GNN coarsening layer (nn_Coarse_layer).

Pipeline (B=2 batches, N=100k nodes, E=800k edges, H=128, C=512 centroids):
  1. fine = relu(concat([x, dist]) @ W + b)                      [B,N,H]
  2. node_avg = segment_mean(fine, cci, C)                       [B,C,H]
  3. coarse edges: group edges by sorted centroid-pair key, drop
     intra-centroid edges, unique keys -> segment_mean(edge_attr) [B,Ec+1,H]

Strategy: the host does the cheap integer index work — pair keys, unique,
argsort — and pads each coarse-edge group to a multiple of PAD=4 edges
(null edges point at an appended all-zero row).  With that layout the big
segment-sum becomes a constant-stationary matmul: fixed shifted aggregation
matrices A_j^T[128, 128] (A_j[s, e] = 1 iff s == j*32 + e//4) reduce 128
streamed edge rows into 32 partial-group rows each, all four subtiles
accumulating into one PSUM tile, with both batches side by side in the
moving operand.  The tensor engine streams with zero per-tile setup work,
PSUM accumulates in fp32, and the host divides by group counts and sums the
split sub-groups (np.add.reduceat) while assembling the output.  The tiny
MLP + 512-way node pooling run on-device as bf16 matmuls with a per-tile
iota/is_equal one-hot for the pooling reduction.  DMA traffic is batched:
GRP=4 superbins ride one 1MB load and one staging store.
"""

import os
import sys

for _p in ("/opt/trn_rl_repo", "/root/.axon_site/_ro/trn_rl_repo"):
    if os.path.isdir(_p) and _p not in sys.path:
        sys.path.append(_p)

import numpy as np
import ml_dtypes

import concourse.bacc as bacc
import concourse.tile as tile
import concourse.mybir as mybir
from concourse.bass_utils import run_bass_kernel_spmd

F32 = mybir.dt.float32
BF16 = mybir.dt.bfloat16
NPBF16 = ml_dtypes.bfloat16

N_CENTROIDS = 512
E_COARSE = N_CENTROIDS * (N_CENTROIDS - 1) // 2
SENTINEL = N_CENTROIDS * N_CENTROIDS
N_CORES = 8

PAD = 4                  # edges per sub-group (aggregation matrix row width)
SUBS = 128 // PAD        # sub-groups produced per matmul (32)
GRP = 4                  # superbins (128 sub-groups each) per DMA transfer
STAGE_F32 = False        # staging dtype; bf16 halves the store traffic

LAST_EXEC_NS = None      # filled when BASS_KERNEL_TRACE=1


def _edge_prep(edge_index, cci, E):
    """Sort edges by coarse-pair slot; pad each slot to a multiple of PAD."""
    sv = cci[edge_index[0]]
    ev = cci[edge_index[1]]
    vmin = np.minimum(sv, ev)
    vmax = np.maximum(sv, ev)
    keys = np.where(sv != ev, vmin * N_CENTROIDS + vmax, SENTINEL)
    uniq, inv = np.unique(keys, return_inverse=True)
    U = int(np.searchsorted(uniq, SENTINEL))        # valid (non-sentinel) slots
    order = np.argsort(inv, kind="stable")
    nval = int(np.searchsorted(inv[order], U))      # edges in valid slots
    order = order[:nval]
    counts = np.bincount(inv[order], minlength=max(U, 1))[:U]
    cum = np.zeros(U + 1, np.int64)
    cum[1:] = np.cumsum(counts)

    subcnt = (counts + PAD - 1) // PAD
    sub_lo = np.zeros(U + 1, np.int64)
    sub_lo[1:] = np.cumsum(subcnt)
    S_sub = int(sub_lo[-1])
    slot_of_sub = np.repeat(np.arange(U, dtype=np.int64), subcnt)
    within = np.arange(S_sub, dtype=np.int64) - sub_lo[slot_of_sub]
    epos = cum[slot_of_sub][:, None] + within[:, None] * PAD + np.arange(PAD)[None, :]
    ok = epos < cum[slot_of_sub + 1][:, None]
    epos_c = np.minimum(epos, max(nval - 1, 0))
    gath = np.where(ok, order[epos_c], E)           # E == appended zero row
    return dict(uniq=uniq, U=U, counts=counts, sub_lo=sub_lo, S_sub=S_sub,
                gath=gath)


def _build_program(nsg, ntp, nt):
    """One SPMD program; all per-core variation comes in through inputs."""
    nc = bacc.Bacc("TRN2", target_bir_lowering=False, debug=False,
                   enable_asserts=False, num_devices=N_CORES)
    stage_dt = F32 if STAGE_F32 else BF16
    ep_in = nc.dram_tensor("ep", [nsg, 128, GRP * 1024], BF16, kind="ExternalInput").ap()
    ag_in = nc.dram_tensor("ag", [128, PAD * 128], BF16, kind="ExternalInput").ap()
    xt_in = nc.dram_tensor("xt", [ntp, 128, 512], BF16, kind="ExternalInput").ap()
    cc_in = nc.dram_tensor("cc", [128, nt], F32, kind="ExternalInput").ap()
    d1_in = nc.dram_tensor("d1", [2, nt * 128], BF16, kind="ExternalInput").ap()
    w_in = nc.dram_tensor("wf", [128, 128], BF16, kind="ExternalInput").ap()
    w2_in = nc.dram_tensor("w2", [2, 256], BF16, kind="ExternalInput").ap()
    io_in = nc.dram_tensor("io", [128, 512], F32, kind="ExternalInput").ap()
    st_out = nc.dram_tensor("stage", [nsg, 128, GRP * 256], stage_dt,
                            kind="ExternalOutput").ap()
    pl_out = nc.dram_tensor("pool", [2, 128, 512], F32, kind="ExternalOutput").ap()

    relu = mybir.ActivationFunctionType.Relu
    iseq = mybir.AluOpType.is_equal

    with tile.TileContext(nc) as tc:
        with tc.tile_pool(name="consts", bufs=1) as consts, \
             tc.tile_pool(name="ep", bufs=4) as ep_pool, \
             tc.tile_pool(name="stg", bufs=4) as stg_pool, \
             tc.tile_pool(name="xp", bufs=4) as x_pool, \
             tc.tile_pool(name="fine", bufs=4) as fine_pool, \
             tc.tile_pool(name="ohn", bufs=4) as ohn_pool, \
             tc.tile_pool(name="pout", bufs=1) as pout_pool, \
             tc.tile_pool(name="eps", bufs=4, space="PSUM") as eps_pool, \
             tc.tile_pool(name="fps", bufs=2, space="PSUM") as fps_pool, \
             tc.tile_pool(name="pps", bufs=1, space="PSUM") as pps_pool:

            agt = consts.tile([128, PAD * 128], BF16, tag="agt")
            nc.sync.dma_start(agt[:], ag_in[:])
            w128 = consts.tile([128, 128], BF16, tag="w128")
            nc.sync.dma_start(w128[:], w_in[:])
            w2b2 = consts.tile([2, 256], BF16, tag="w2b2")
            nc.sync.dma_start(w2b2[:], w2_in[:])
            iot = consts.tile([128, 512], F32, tag="iot")
            nc.sync.dma_start(iot[:], io_in[:])
            cct = consts.tile([128, nt], F32, tag="cct")
            nc.sync.dma_start(cct[:], cc_in[:])
            d1t = consts.tile([2, nt * 128], BF16, tag="d1t")
            nc.sync.dma_start(d1t[:], d1_in[:])

            pool_ps = [pps_pool.tile([128, 512], F32, tag=f"pp{b}", name=f"pool_ps{b}")
                       for b in range(2)]

            def edge_group(grp):
                ept = ep_pool.tile([128, GRP * 1024], BF16, tag="ept",
                                   name=f"ept{grp}")
                nc.sync.dma_start(ept[:], ep_in[grp])
                stg = stg_pool.tile([128, GRP * 256], stage_dt, tag="stg",
                                    name=f"stg{grp}")
                for g in range(GRP):
                    eps = eps_pool.tile([128, 256], F32, tag="eps",
                                        name=f"eps{grp}_{g}")
                    for j in range(PAD):
                        nc.tensor.matmul(
                            eps[:], agt[:, j * 128:(j + 1) * 128],
                            ept[:, g * 1024 + j * 256: g * 1024 + (j + 1) * 256],
                            start=(j == 0), stop=(j == PAD - 1))
                    dst = stg[:, g * 256:(g + 1) * 256]
                    if g % 2 == 0:
                        nc.vector.tensor_copy(dst, eps[:])
                    else:
                        nc.scalar.copy(dst, eps[:])
                nc.gpsimd.dma_start(st_out[grp], stg[:])

            def node_pair(tp):
                xtt = x_pool.tile([128, 512], BF16, tag="xt", name=f"xt{tp}")
                nc.sync.dma_start(xtt[:], xt_in[tp])
                for half in range(2):
                    t = tp * 2 + half
                    ohn = ohn_pool.tile([128, 512], BF16, tag="ohn", name=f"ohn{t}")
                    nc.vector.tensor_scalar(ohn[:], iot[:], cct[:, t:t + 1], None, iseq)
                    fps = fps_pool.tile([128, 256], F32, tag="fps", name=f"fps{t}")
                    # d-term + bias first (writes all elements), then x@W halves
                    nc.tensor.matmul(fps[:], d1t[:, t * 128:(t + 1) * 128], w2b2[:],
                                     start=True, stop=False)
                    for bch in range(2):
                        nc.tensor.matmul(
                            fps[:, bch * 128:(bch + 1) * 128],
                            xtt[:, bch * 256 + half * 128: bch * 256 + (half + 1) * 128],
                            w128[:], start=False, stop=True)
                    fsb = fine_pool.tile([128, 256], BF16, tag="fsb", name=f"fsb{t}")
                    nc.scalar.activation(fsb[:], fps[:], relu)
                    for bch in range(2):
                        nc.tensor.matmul(pool_ps[bch][:],
                                         fsb[:, bch * 128:(bch + 1) * 128], ohn[:],
                                         start=(t == 0), stop=(t == nt - 1))

            for i in range(max(nsg, ntp)):
                if i < nsg:
                    edge_group(i)
                if i < ntp:
                    node_pair(i)

            for bch in range(2):
                po = pout_pool.tile([128, 512], F32, tag=f"po{bch}", name=f"po{bch}")
                nc.vector.tensor_copy(po[:], pool_ps[bch][:])
                nc.sync.dma_start(pl_out[bch], po[:])

    nc.compile()
    return nc


def kernel(x, edge_index, edge_attr, scale=None, closest_centroid_indices=None,
           distances=None, W=None, b=None, **_unused):
    global LAST_EXEC_NS
    x_np = np.asarray(x, dtype=np.float32)
    ei = np.asarray(edge_index)
    ea = np.asarray(edge_attr, dtype=np.float32)
    cci_in = closest_centroid_indices
    cci = np.asarray(cci_in).astype(np.int64)
    dist = np.asarray(distances, dtype=np.float32)
    W_np = np.asarray(W, dtype=np.float32)
    b_np = np.asarray(b, dtype=np.float32)

    B, N, H = x_np.shape
    E = ei.shape[1]
    assert H == 128 and B == 2 and N % N_CORES == 0

    # ---------------- host index prep ----------------
    em = _edge_prep(ei.astype(np.int64), cci, E)
    S_sub = em["S_sub"]
    per_grp = GRP * 128                                 # sub-groups per DMA group
    nsg = max((S_sub + N_CORES * per_grp - 1) // (N_CORES * per_grp), 1)
    spc = nsg * per_grp                                 # sub-groups per core
    npc = N // N_CORES
    nt = (npc + 127) // 128
    if nt % 2:
        nt += 1
    ntp = nt // 2
    npc_pad = nt * 128

    ea_ext = np.concatenate([ea, np.zeros((2, 1, H), np.float32)], 1)

    iota512 = np.broadcast_to(np.arange(512, dtype=np.float32), (128, 512)).copy()
    # A_j[s, e] = 1 iff s == j*32 + e//PAD; stored as A_j^T [e, s]
    e_over = np.arange(128)[:, None] // PAD
    aseg = np.concatenate([(e_over + j * SUBS == np.arange(128)[None, :])
                           for j in range(PAD)], 1).astype(NPBF16)  # [128,PAD*128]
    wfull = W_np[:128].astype(NPBF16)                            # [128,128]
    w2row = np.concatenate([W_np[128][None, :], b_np[None, :]], 0)  # [2,128]
    w2b2 = np.concatenate([w2row, w2row], 1).astype(NPBF16)      # [2,256]

    in_maps = []
    for c in range(N_CORES):
        lo_s, hi_s = c * spc, min((c + 1) * spc, S_sub)
        gath = np.full((spc, PAD), E, np.int64)
        if hi_s > lo_s:
            gath[:hi_s - lo_s] = em["gath"][lo_s:hi_s]
        g = ea_ext[:, gath.reshape(-1), :]              # [2, spc*PAD, 128]
        g = g.reshape(2, nsg, GRP, PAD, SUBS, PAD, 128).transpose(1, 4, 5, 2, 3, 0, 6)
        ep = np.ascontiguousarray(g.reshape(nsg, 128, GRP * 1024)).astype(NPBF16)

        lo = c * npc
        xs = np.zeros((2, npc_pad, 128), np.float32)
        xs[:, :npc] = x_np[:, lo:lo + npc]
        xt = np.ascontiguousarray(
            xs.reshape(2, ntp, 256, 128).transpose(1, 3, 0, 2).reshape(ntp, 128, 512)
        ).astype(NPBF16)
        cc_pad = np.full(npc_pad, -1.0, np.float32)
        cc_pad[:npc] = cci[lo:lo + npc].astype(np.float32)
        cc = np.ascontiguousarray(cc_pad.reshape(nt, 128).T)    # [128, nt]
        d1 = np.ones((2, npc_pad), np.float32)
        d1[0, :] = 0.0
        d1[0, :npc] = dist[lo:lo + npc]

        in_maps.append({"ep": ep, "ag": aseg, "xt": xt, "cc": cc,
                        "d1": d1.astype(NPBF16), "wf": wfull, "w2": w2b2,
                        "io": iota512})

    # ---------------- build + run ----------------
    nc = _build_program(nsg, ntp, nt)
    trace = os.environ.get("BASS_KERNEL_TRACE", "0") == "1"
    if trace:
        try:
            import profhook
            profhook.install()
        except Exception:
            trace = False
    res = run_bass_kernel_spmd(nc, in_maps, core_ids=list(range(N_CORES)),
                               trace=trace)
    LAST_EXEC_NS = res.exec_time_ns

    # ---------------- host postprocessing ----------------
    pool_sum = np.zeros((2, 128, 512), np.float64)
    for c in range(N_CORES):
        pool_sum += res.results[c]["pool"]
    pool_sum = pool_sum.astype(np.float32)
    ncnt = np.bincount(cci, minlength=N_CENTROIDS).astype(np.float32)
    node_avg = pool_sum.transpose(0, 2, 1) / np.clip(ncnt, 1.0, None)[None, :, None]

    # coarse edge attrs: gather real sub-group rows, reduce per slot
    U = em["U"]
    chunks = []
    for c in range(N_CORES):
        lo_s, hi_s = c * spc, min((c + 1) * spc, S_sub)
        if hi_s > lo_s:
            st = res.results[c]["stage"].astype(np.float32)
            st = st.reshape(nsg, 128, GRP, 256).transpose(0, 2, 1, 3).reshape(spc, 256)
            chunks.append(st[:hi_s - lo_s])
    out_attr = np.zeros((2, E_COARSE + 1, 128), np.float32)
    if U > 0:
        rows = np.concatenate(chunks, 0)                         # [S_sub, 256]
        esums = np.add.reduceat(rows, em["sub_lo"][:U], axis=0)  # [U, 256]
        emeans = esums / np.clip(em["counts"].astype(np.float32), 1.0, None)[:, None]
        out_attr[0, :U] = emeans[:, :128]
        out_attr[1, :U] = emeans[:, 128:]

    uniq = em["uniq"]
    uniq_pad = np.full(E_COARSE + 1, SENTINEL, np.int64)
    uniq_pad[:min(len(uniq), E_COARSE + 1)] = uniq[:E_COARSE + 1]
    idx_dt = np.asarray(cci_in).dtype
    if idx_dt.kind not in "iu":
        idx_dt = np.dtype(np.int32)
    ce = np.where(uniq_pad < SENTINEL,
                  np.stack([uniq_pad // N_CENTROIDS, uniq_pad % N_CENTROIDS]),
                  -1).astype(idx_dt)

    return (node_avg, out_attr, ce, np.asarray(cci_in), np.asarray(distances))


# revision 14
# speedup vs baseline: 1.0030x; 1.0030x over previous
"""Trainium2 Bass kernel for the GNN coarsening layer (nn_Coarse_layer).

Pipeline (B=2 batches, N=100k nodes, E=800k edges, H=128, C=512 centroids):
  1. fine = relu(concat([x, dist]) @ W + b)                      [B,N,H]
  2. node_avg = segment_mean(fine, cci, C)                       [B,C,H]
  3. coarse edges: group edges by sorted centroid-pair key, drop
     intra-centroid edges, unique keys -> segment_mean(edge_attr) [B,Ec+1,H]

Strategy: the host does the cheap integer index work — pair keys, unique,
argsort — and pads each coarse-edge group to a multiple of PAD=4 edges
(null edges point at an appended all-zero row).  With that layout the big
segment-sum becomes a constant-stationary matmul: fixed shifted aggregation
matrices A_j^T[128, 128] (A_j[s, e] = 1 iff s == j*32 + e//4) reduce 128
streamed edge rows into 32 partial-group rows each, all four subtiles
accumulating into one PSUM tile, with both batches side by side in the
moving operand.  The tensor engine streams with zero per-tile setup work,
PSUM accumulates in fp32, and the host divides by group counts and sums the
split sub-groups (np.add.reduceat) while assembling the output.  The tiny
MLP + 512-way node pooling run on-device as bf16 matmuls with a per-tile
iota/is_equal one-hot for the pooling reduction.  DMA traffic is batched:
GRP=4 superbins ride one 1MB load and one staging store.
"""

import os
import sys

for _p in ("/opt/trn_rl_repo", "/root/.axon_site/_ro/trn_rl_repo"):
    if os.path.isdir(_p) and _p not in sys.path:
        sys.path.append(_p)

import numpy as np
import ml_dtypes

import concourse.bacc as bacc
import concourse.tile as tile
import concourse.mybir as mybir
from concourse.bass_utils import run_bass_kernel_spmd

F32 = mybir.dt.float32
BF16 = mybir.dt.bfloat16
NPBF16 = ml_dtypes.bfloat16

N_CENTROIDS = 512
E_COARSE = N_CENTROIDS * (N_CENTROIDS - 1) // 2
SENTINEL = N_CENTROIDS * N_CENTROIDS
N_CORES = 8

PAD = 4                  # edges per sub-group (aggregation matrix row width)
SUBS = 128 // PAD        # sub-groups produced per matmul (32)
GRP = 4                  # superbins (128 sub-groups each) per DMA transfer
STAGE_F32 = False        # staging dtype; bf16 halves the store traffic

LAST_EXEC_NS = None      # filled when BASS_KERNEL_TRACE=1


def _edge_prep(edge_index, cci, E):
    """Sort edges by coarse-pair slot; pad each slot to a multiple of PAD."""
    sv = cci[edge_index[0]]
    ev = cci[edge_index[1]]
    vmin = np.minimum(sv, ev)
    vmax = np.maximum(sv, ev)
    keys = np.where(sv != ev, vmin * N_CENTROIDS + vmax, SENTINEL)
    uniq, inv = np.unique(keys, return_inverse=True)
    U = int(np.searchsorted(uniq, SENTINEL))        # valid (non-sentinel) slots
    order = np.argsort(inv, kind="stable")
    nval = int(np.searchsorted(inv[order], U))      # edges in valid slots
    order = order[:nval]
    counts = np.bincount(inv[order], minlength=max(U, 1))[:U]
    cum = np.zeros(U + 1, np.int64)
    cum[1:] = np.cumsum(counts)

    subcnt = (counts + PAD - 1) // PAD
    sub_lo = np.zeros(U + 1, np.int64)
    sub_lo[1:] = np.cumsum(subcnt)
    S_sub = int(sub_lo[-1])
    slot_of_sub = np.repeat(np.arange(U, dtype=np.int64), subcnt)
    within = np.arange(S_sub, dtype=np.int64) - sub_lo[slot_of_sub]
    epos = cum[slot_of_sub][:, None] + within[:, None] * PAD + np.arange(PAD)[None, :]
    ok = epos < cum[slot_of_sub + 1][:, None]
    epos_c = np.minimum(epos, max(nval - 1, 0))
    gath = np.where(ok, order[epos_c], E)           # E == appended zero row
    return dict(uniq=uniq, U=U, counts=counts, sub_lo=sub_lo, S_sub=S_sub,
                gath=gath)


def _build_program(nsg, ntp, nt):
    """One SPMD program; all per-core variation comes in through inputs."""
    nc = bacc.Bacc("TRN2", target_bir_lowering=False, debug=False,
                   enable_asserts=False, num_devices=N_CORES)
    stage_dt = F32 if STAGE_F32 else BF16
    ep_in = nc.dram_tensor("ep", [nsg, 128, GRP * 1024], BF16, kind="ExternalInput").ap()
    ag_in = nc.dram_tensor("ag", [128, PAD * 128], BF16, kind="ExternalInput").ap()
    xt_in = nc.dram_tensor("xt", [ntp, 128, 512], BF16, kind="ExternalInput").ap()
    cc_in = nc.dram_tensor("cc", [128, nt], F32, kind="ExternalInput").ap()
    d1_in = nc.dram_tensor("d1", [2, nt * 128], BF16, kind="ExternalInput").ap()
    w_in = nc.dram_tensor("wf", [128, 128], BF16, kind="ExternalInput").ap()
    w2_in = nc.dram_tensor("w2", [2, 256], BF16, kind="ExternalInput").ap()
    io_in = nc.dram_tensor("io", [128, 512], F32, kind="ExternalInput").ap()
    st_out = nc.dram_tensor("stage", [nsg, 128, GRP * 256], stage_dt,
                            kind="ExternalOutput").ap()
    pl_out = nc.dram_tensor("pool", [2, 128, 512], F32, kind="ExternalOutput").ap()

    relu = mybir.ActivationFunctionType.Relu
    iseq = mybir.AluOpType.is_equal

    with tile.TileContext(nc) as tc:
        with tc.tile_pool(name="consts", bufs=1) as consts, \
             tc.tile_pool(name="ep", bufs=6) as ep_pool, \
             tc.tile_pool(name="stg", bufs=4) as stg_pool, \
             tc.tile_pool(name="xp", bufs=6) as x_pool, \
             tc.tile_pool(name="fine", bufs=4) as fine_pool, \
             tc.tile_pool(name="ohn", bufs=4) as ohn_pool, \
             tc.tile_pool(name="pout", bufs=1) as pout_pool, \
             tc.tile_pool(name="eps", bufs=4, space="PSUM") as eps_pool, \
             tc.tile_pool(name="fps", bufs=2, space="PSUM") as fps_pool, \
             tc.tile_pool(name="pps", bufs=1, space="PSUM") as pps_pool:

            agt = consts.tile([128, PAD * 128], BF16, tag="agt")
            nc.sync.dma_start(agt[:], ag_in[:])
            w128 = consts.tile([128, 128], BF16, tag="w128")
            nc.sync.dma_start(w128[:], w_in[:])
            w2b2 = consts.tile([2, 256], BF16, tag="w2b2")
            nc.sync.dma_start(w2b2[:], w2_in[:])
            iot = consts.tile([128, 512], F32, tag="iot")
            nc.sync.dma_start(iot[:], io_in[:])
            cct = consts.tile([128, nt], F32, tag="cct")
            nc.sync.dma_start(cct[:], cc_in[:])
            d1t = consts.tile([2, nt * 128], BF16, tag="d1t")
            nc.sync.dma_start(d1t[:], d1_in[:])

            pool_ps = [pps_pool.tile([128, 512], F32, tag=f"pp{b}", name=f"pool_ps{b}")
                       for b in range(2)]

            def edge_group(grp):
                ept = ep_pool.tile([128, GRP * 1024], BF16, tag="ept",
                                   name=f"ept{grp}")
                nc.sync.dma_start(ept[:], ep_in[grp])
                stg = stg_pool.tile([128, GRP * 256], stage_dt, tag="stg",
                                    name=f"stg{grp}")
                for g in range(GRP):
                    eps = eps_pool.tile([128, 256], F32, tag="eps",
                                        name=f"eps{grp}_{g}")
                    for j in range(PAD):
                        nc.tensor.matmul(
                            eps[:], agt[:, j * 128:(j + 1) * 128],
                            ept[:, g * 1024 + j * 256: g * 1024 + (j + 1) * 256],
                            start=(j == 0), stop=(j == PAD - 1))
                    dst = stg[:, g * 256:(g + 1) * 256]
                    if g % 2 == 0:
                        nc.vector.tensor_copy(dst, eps[:])
                    else:
                        nc.scalar.copy(dst, eps[:])
                nc.gpsimd.dma_start(st_out[grp], stg[:])

            def node_pair(tp):
                xtt = x_pool.tile([128, 512], BF16, tag="xt", name=f"xt{tp}")
                nc.sync.dma_start(xtt[:], xt_in[tp])
                for half in range(2):
                    t = tp * 2 + half
                    ohn = ohn_pool.tile([128, 512], BF16, tag="ohn", name=f"ohn{t}")
                    nc.vector.tensor_scalar(ohn[:], iot[:], cct[:, t:t + 1], None, iseq)
                    fps = fps_pool.tile([128, 256], F32, tag="fps", name=f"fps{t}")
                    # d-term + bias first (writes all elements), then x@W halves
                    nc.tensor.matmul(fps[:], d1t[:, t * 128:(t + 1) * 128], w2b2[:],
                                     start=True, stop=False)
                    for bch in range(2):
                        nc.tensor.matmul(
                            fps[:, bch * 128:(bch + 1) * 128],
                            xtt[:, bch * 256 + half * 128: bch * 256 + (half + 1) * 128],
                            w128[:], start=False, stop=True)
                    fsb = fine_pool.tile([128, 256], BF16, tag="fsb", name=f"fsb{t}")
                    nc.scalar.activation(fsb[:], fps[:], relu)
                    for bch in range(2):
                        nc.tensor.matmul(pool_ps[bch][:],
                                         fsb[:, bch * 128:(bch + 1) * 128], ohn[:],
                                         start=(t == 0), stop=(t == nt - 1))

            for i in range(max(nsg, ntp)):
                if i < nsg:
                    edge_group(i)
                if i < ntp:
                    node_pair(i)

            for bch in range(2):
                po = pout_pool.tile([128, 512], F32, tag=f"po{bch}", name=f"po{bch}")
                nc.vector.tensor_copy(po[:], pool_ps[bch][:])
                nc.sync.dma_start(pl_out[bch], po[:])

    nc.compile()
    return nc


def kernel(x, edge_index, edge_attr, scale=None, closest_centroid_indices=None,
           distances=None, W=None, b=None, **_unused):
    global LAST_EXEC_NS
    x_np = np.asarray(x, dtype=np.float32)
    ei = np.asarray(edge_index)
    ea = np.asarray(edge_attr, dtype=np.float32)
    cci_in = closest_centroid_indices
    cci = np.asarray(cci_in).astype(np.int64)
    dist = np.asarray(distances, dtype=np.float32)
    W_np = np.asarray(W, dtype=np.float32)
    b_np = np.asarray(b, dtype=np.float32)

    B, N, H = x_np.shape
    E = ei.shape[1]
    assert H == 128 and B == 2 and N % N_CORES == 0

    # ---------------- host index prep ----------------
    em = _edge_prep(ei.astype(np.int64), cci, E)
    S_sub = em["S_sub"]
    per_grp = GRP * 128                                 # sub-groups per DMA group
    nsg = max((S_sub + N_CORES * per_grp - 1) // (N_CORES * per_grp), 1)
    spc = nsg * per_grp                                 # sub-groups per core
    npc = N // N_CORES
    nt = (npc + 127) // 128
    if nt % 2:
        nt += 1
    ntp = nt // 2
    npc_pad = nt * 128

    ea_ext = np.concatenate([ea, np.zeros((2, 1, H), np.float32)], 1)

    iota512 = np.broadcast_to(np.arange(512, dtype=np.float32), (128, 512)).copy()
    # A_j[s, e] = 1 iff s == j*32 + e//PAD; stored as A_j^T [e, s]
    e_over = np.arange(128)[:, None] // PAD
    aseg = np.concatenate([(e_over + j * SUBS == np.arange(128)[None, :])
                           for j in range(PAD)], 1).astype(NPBF16)  # [128,PAD*128]
    wfull = W_np[:128].astype(NPBF16)                            # [128,128]
    w2row = np.concatenate([W_np[128][None, :], b_np[None, :]], 0)  # [2,128]
    w2b2 = np.concatenate([w2row, w2row], 1).astype(NPBF16)      # [2,256]

    in_maps = []
    for c in range(N_CORES):
        lo_s, hi_s = c * spc, min((c + 1) * spc, S_sub)
        gath = np.full((spc, PAD), E, np.int64)
        if hi_s > lo_s:
            gath[:hi_s - lo_s] = em["gath"][lo_s:hi_s]
        g = ea_ext[:, gath.reshape(-1), :]              # [2, spc*PAD, 128]
        g = g.reshape(2, nsg, GRP, PAD, SUBS, PAD, 128).transpose(1, 4, 5, 2, 3, 0, 6)
        ep = np.ascontiguousarray(g.reshape(nsg, 128, GRP * 1024)).astype(NPBF16)

        lo = c * npc
        xs = np.zeros((2, npc_pad, 128), np.float32)
        xs[:, :npc] = x_np[:, lo:lo + npc]
        xt = np.ascontiguousarray(
            xs.reshape(2, ntp, 256, 128).transpose(1, 3, 0, 2).reshape(ntp, 128, 512)
        ).astype(NPBF16)
        cc_pad = np.full(npc_pad, -1.0, np.float32)
        cc_pad[:npc] = cci[lo:lo + npc].astype(np.float32)
        cc = np.ascontiguousarray(cc_pad.reshape(nt, 128).T)    # [128, nt]
        d1 = np.ones((2, npc_pad), np.float32)
        d1[0, :] = 0.0
        d1[0, :npc] = dist[lo:lo + npc]

        in_maps.append({"ep": ep, "ag": aseg, "xt": xt, "cc": cc,
                        "d1": d1.astype(NPBF16), "wf": wfull, "w2": w2b2,
                        "io": iota512})

    # ---------------- build + run ----------------
    nc = _build_program(nsg, ntp, nt)
    trace = os.environ.get("BASS_KERNEL_TRACE", "0") == "1"
    if trace:
        try:
            import profhook
            profhook.install()
        except Exception:
            trace = False
    res = run_bass_kernel_spmd(nc, in_maps, core_ids=list(range(N_CORES)),
                               trace=trace)
    LAST_EXEC_NS = res.exec_time_ns

    # ---------------- host postprocessing ----------------
    pool_sum = np.zeros((2, 128, 512), np.float64)
    for c in range(N_CORES):
        pool_sum += res.results[c]["pool"]
    pool_sum = pool_sum.astype(np.float32)
    ncnt = np.bincount(cci, minlength=N_CENTROIDS).astype(np.float32)
    node_avg = pool_sum.transpose(0, 2, 1) / np.clip(ncnt, 1.0, None)[None, :, None]

    # coarse edge attrs: gather real sub-group rows, reduce per slot
    U = em["U"]
    chunks = []
    for c in range(N_CORES):
        lo_s, hi_s = c * spc, min((c + 1) * spc, S_sub)
        if hi_s > lo_s:
            st = res.results[c]["stage"].astype(np.float32)
            st = st.reshape(nsg, 128, GRP, 256).transpose(0, 2, 1, 3).reshape(spc, 256)
            chunks.append(st[:hi_s - lo_s])
    out_attr = np.zeros((2, E_COARSE + 1, 128), np.float32)
    if U > 0:
        rows = np.concatenate(chunks, 0)                         # [S_sub, 256]
        esums = np.add.reduceat(rows, em["sub_lo"][:U], axis=0)  # [U, 256]
        emeans = esums / np.clip(em["counts"].astype(np.float32), 1.0, None)[:, None]
        out_attr[0, :U] = emeans[:, :128]
        out_attr[1, :U] = emeans[:, 128:]

    uniq = em["uniq"]
    uniq_pad = np.full(E_COARSE + 1, SENTINEL, np.int64)
    uniq_pad[:min(len(uniq), E_COARSE + 1)] = uniq[:E_COARSE + 1]
    idx_dt = np.asarray(cci_in).dtype
    if idx_dt.kind not in "iu":
        idx_dt = np.dtype(np.int32)
    ce = np.where(uniq_pad < SENTINEL,
                  np.stack([uniq_pad // N_CENTROIDS, uniq_pad % N_CENTROIDS]),
                  -1).astype(idx_dt)

    return (node_avg, out_attr, ce, np.asarray(cci_in), np.asarray(distances))


# revision 15
# speedup vs baseline: 1.0117x; 1.0087x over previous
"""Trainium2 Bass kernel for the GNN coarsening layer (nn_Coarse_layer).

Pipeline (B=2 batches, N=100k nodes, E=800k edges, H=128, C=512 centroids):
  1. fine = relu(concat([x, dist]) @ W + b)                      [B,N,H]
  2. node_avg = segment_mean(fine, cci, C)                       [B,C,H]
  3. coarse edges: group edges by sorted centroid-pair key, drop
     intra-centroid edges, unique keys -> segment_mean(edge_attr) [B,Ec+1,H]

Strategy: the host does the cheap integer index work — pair keys, unique,
argsort — and pads each coarse-edge group to a multiple of PAD=4 edges
(null edges point at an appended all-zero row).  With that layout the big
segment-sum becomes a constant-stationary matmul: fixed shifted aggregation
matrices A_j^T[128, 128] (A_j[s, e] = 1 iff s == j*32 + e//4) reduce 128
streamed edge rows into 32 partial-group rows each, all four subtiles
accumulating into one PSUM tile, with both batches side by side in the
moving operand.  The tensor engine streams with zero per-tile setup work,
PSUM accumulates in fp32, and the host divides by group counts and sums the
split sub-groups (np.add.reduceat) while assembling the output.  The tiny
MLP + 512-way node pooling run on-device as bf16 matmuls with a per-tile
iota/is_equal one-hot for the pooling reduction.  DMA traffic is batched:
GRP=4 superbins ride one 1MB load and one staging store.
"""

import os
import sys

for _p in ("/opt/trn_rl_repo", "/root/.axon_site/_ro/trn_rl_repo"):
    if os.path.isdir(_p) and _p not in sys.path:
        sys.path.append(_p)

import numpy as np
import ml_dtypes

import concourse.bacc as bacc
import concourse.tile as tile
import concourse.mybir as mybir
from concourse.bass_utils import run_bass_kernel_spmd

F32 = mybir.dt.float32
BF16 = mybir.dt.bfloat16
NPBF16 = ml_dtypes.bfloat16

N_CENTROIDS = 512
E_COARSE = N_CENTROIDS * (N_CENTROIDS - 1) // 2
SENTINEL = N_CENTROIDS * N_CENTROIDS
N_CORES = 8

PAD = 4                  # edges per sub-group (aggregation matrix row width)
SUBS = 128 // PAD        # sub-groups produced per matmul (32)
GRP = 4                  # superbins (128 sub-groups each) per DMA transfer
STAGE_F32 = False        # staging dtype; bf16 halves the store traffic

LAST_EXEC_NS = None      # filled when BASS_KERNEL_TRACE=1


def _edge_prep(edge_index, cci, E):
    """Sort edges by coarse-pair slot; pad each slot to a multiple of PAD."""
    sv = cci[edge_index[0]]
    ev = cci[edge_index[1]]
    vmin = np.minimum(sv, ev)
    vmax = np.maximum(sv, ev)
    keys = np.where(sv != ev, vmin * N_CENTROIDS + vmax, SENTINEL)
    uniq, inv = np.unique(keys, return_inverse=True)
    U = int(np.searchsorted(uniq, SENTINEL))        # valid (non-sentinel) slots
    order = np.argsort(inv, kind="stable")
    nval = int(np.searchsorted(inv[order], U))      # edges in valid slots
    order = order[:nval]
    counts = np.bincount(inv[order], minlength=max(U, 1))[:U]
    cum = np.zeros(U + 1, np.int64)
    cum[1:] = np.cumsum(counts)

    subcnt = (counts + PAD - 1) // PAD
    sub_lo = np.zeros(U + 1, np.int64)
    sub_lo[1:] = np.cumsum(subcnt)
    S_sub = int(sub_lo[-1])
    slot_of_sub = np.repeat(np.arange(U, dtype=np.int64), subcnt)
    within = np.arange(S_sub, dtype=np.int64) - sub_lo[slot_of_sub]
    epos = cum[slot_of_sub][:, None] + within[:, None] * PAD + np.arange(PAD)[None, :]
    ok = epos < cum[slot_of_sub + 1][:, None]
    epos_c = np.minimum(epos, max(nval - 1, 0))
    gath = np.where(ok, order[epos_c], E)           # E == appended zero row
    return dict(uniq=uniq, U=U, counts=counts, sub_lo=sub_lo, S_sub=S_sub,
                gath=gath)


def _build_program(nsg, ntp, nt):
    """One SPMD program; all per-core variation comes in through inputs."""
    nc = bacc.Bacc("TRN2", target_bir_lowering=False, debug=False,
                   enable_asserts=False, num_devices=N_CORES)
    stage_dt = F32 if STAGE_F32 else BF16
    ep_in = nc.dram_tensor("ep", [nsg, 128, GRP * 1024], BF16, kind="ExternalInput").ap()
    ag_in = nc.dram_tensor("ag", [128, PAD * 128], BF16, kind="ExternalInput").ap()
    xt_in = nc.dram_tensor("xt", [ntp, 128, 512], BF16, kind="ExternalInput").ap()
    cc_in = nc.dram_tensor("cc", [128, nt], F32, kind="ExternalInput").ap()
    d1_in = nc.dram_tensor("d1", [2, nt * 128], BF16, kind="ExternalInput").ap()
    w_in = nc.dram_tensor("wf", [128, 128], BF16, kind="ExternalInput").ap()
    w2_in = nc.dram_tensor("w2", [2, 256], BF16, kind="ExternalInput").ap()
    io_in = nc.dram_tensor("io", [128, 512], F32, kind="ExternalInput").ap()
    st_out = nc.dram_tensor("stage", [nsg, 128, GRP * 256], stage_dt,
                            kind="ExternalOutput").ap()
    pl_out = nc.dram_tensor("pool", [2, 128, 512], F32, kind="ExternalOutput").ap()

    relu = mybir.ActivationFunctionType.Relu
    iseq = mybir.AluOpType.is_equal

    with tile.TileContext(nc) as tc:
        with tc.tile_pool(name="consts", bufs=1) as consts, \
             tc.tile_pool(name="ep", bufs=6) as ep_pool, \
             tc.tile_pool(name="stg", bufs=4) as stg_pool, \
             tc.tile_pool(name="xp", bufs=6) as x_pool, \
             tc.tile_pool(name="fine", bufs=4) as fine_pool, \
             tc.tile_pool(name="ohn", bufs=4) as ohn_pool, \
             tc.tile_pool(name="pout", bufs=1) as pout_pool, \
             tc.tile_pool(name="eps", bufs=5, space="PSUM") as eps_pool, \
             tc.tile_pool(name="fps", bufs=1, space="PSUM") as fps_pool, \
             tc.tile_pool(name="pps", bufs=1, space="PSUM") as pps_pool:

            agt = consts.tile([128, PAD * 128], BF16, tag="agt")
            nc.sync.dma_start(agt[:], ag_in[:])
            w128 = consts.tile([128, 128], BF16, tag="w128")
            nc.sync.dma_start(w128[:], w_in[:])
            w2b2 = consts.tile([2, 256], BF16, tag="w2b2")
            nc.sync.dma_start(w2b2[:], w2_in[:])
            iot = consts.tile([128, 512], F32, tag="iot")
            nc.sync.dma_start(iot[:], io_in[:])
            cct = consts.tile([128, nt], F32, tag="cct")
            nc.sync.dma_start(cct[:], cc_in[:])
            d1t = consts.tile([2, nt * 128], BF16, tag="d1t")
            nc.sync.dma_start(d1t[:], d1_in[:])

            pool_ps = [pps_pool.tile([128, 512], F32, tag=f"pp{b}", name=f"pool_ps{b}")
                       for b in range(2)]

            def edge_group(grp):
                ept = ep_pool.tile([128, GRP * 1024], BF16, tag="ept",
                                   name=f"ept{grp}")
                nc.sync.dma_start(ept[:], ep_in[grp])
                stg = stg_pool.tile([128, GRP * 256], stage_dt, tag="stg",
                                    name=f"stg{grp}")
                for g in range(GRP):
                    eps = eps_pool.tile([128, 256], F32, tag="eps",
                                        name=f"eps{grp}_{g}")
                    for j in range(PAD):
                        nc.tensor.matmul(
                            eps[:], agt[:, j * 128:(j + 1) * 128],
                            ept[:, g * 1024 + j * 256: g * 1024 + (j + 1) * 256],
                            start=(j == 0), stop=(j == PAD - 1))
                    dst = stg[:, g * 256:(g + 1) * 256]
                    if g % 2 == 0:
                        nc.vector.tensor_copy(dst, eps[:])
                    else:
                        nc.scalar.copy(dst, eps[:])
                nc.gpsimd.dma_start(st_out[grp], stg[:])

            def node_pair(tp):
                xtt = x_pool.tile([128, 512], BF16, tag="xt", name=f"xt{tp}")
                nc.sync.dma_start(xtt[:], xt_in[tp])
                for half in range(2):
                    t = tp * 2 + half
                    ohn = ohn_pool.tile([128, 512], BF16, tag="ohn", name=f"ohn{t}")
                    nc.vector.tensor_scalar(ohn[:], iot[:], cct[:, t:t + 1], None, iseq)
                    fps = fps_pool.tile([128, 256], F32, tag="fps", name=f"fps{t}")
                    # d-term + bias first (writes all elements), then x@W halves
                    nc.tensor.matmul(fps[:], d1t[:, t * 128:(t + 1) * 128], w2b2[:],
                                     start=True, stop=False)
                    for bch in range(2):
                        nc.tensor.matmul(
                            fps[:, bch * 128:(bch + 1) * 128],
                            xtt[:, bch * 256 + half * 128: bch * 256 + (half + 1) * 128],
                            w128[:], start=False, stop=True)
                    fsb = fine_pool.tile([128, 256], BF16, tag="fsb", name=f"fsb{t}")
                    nc.scalar.activation(fsb[:], fps[:], relu)
                    for bch in range(2):
                        nc.tensor.matmul(pool_ps[bch][:],
                                         fsb[:, bch * 128:(bch + 1) * 128], ohn[:],
                                         start=(t == 0), stop=(t == nt - 1))

            for i in range(max(nsg, ntp)):
                if i < nsg:
                    edge_group(i)
                if i < ntp:
                    node_pair(i)

            for bch in range(2):
                po = pout_pool.tile([128, 512], F32, tag=f"po{bch}", name=f"po{bch}")
                nc.vector.tensor_copy(po[:], pool_ps[bch][:])
                nc.sync.dma_start(pl_out[bch], po[:])

    nc.compile()
    return nc


def kernel(x, edge_index, edge_attr, scale=None, closest_centroid_indices=None,
           distances=None, W=None, b=None, **_unused):
    global LAST_EXEC_NS
    x_np = np.asarray(x, dtype=np.float32)
    ei = np.asarray(edge_index)
    ea = np.asarray(edge_attr, dtype=np.float32)
    cci_in = closest_centroid_indices
    cci = np.asarray(cci_in).astype(np.int64)
    dist = np.asarray(distances, dtype=np.float32)
    W_np = np.asarray(W, dtype=np.float32)
    b_np = np.asarray(b, dtype=np.float32)

    B, N, H = x_np.shape
    E = ei.shape[1]
    assert H == 128 and B == 2 and N % N_CORES == 0

    # ---------------- host index prep ----------------
    em = _edge_prep(ei.astype(np.int64), cci, E)
    S_sub = em["S_sub"]
    per_grp = GRP * 128                                 # sub-groups per DMA group
    nsg = max((S_sub + N_CORES * per_grp - 1) // (N_CORES * per_grp), 1)
    spc = nsg * per_grp                                 # sub-groups per core
    npc = N // N_CORES
    nt = (npc + 127) // 128
    if nt % 2:
        nt += 1
    ntp = nt // 2
    npc_pad = nt * 128

    ea_ext = np.concatenate([ea, np.zeros((2, 1, H), np.float32)], 1)

    iota512 = np.broadcast_to(np.arange(512, dtype=np.float32), (128, 512)).copy()
    # A_j[s, e] = 1 iff s == j*32 + e//PAD; stored as A_j^T [e, s]
    e_over = np.arange(128)[:, None] // PAD
    aseg = np.concatenate([(e_over + j * SUBS == np.arange(128)[None, :])
                           for j in range(PAD)], 1).astype(NPBF16)  # [128,PAD*128]
    wfull = W_np[:128].astype(NPBF16)                            # [128,128]
    w2row = np.concatenate([W_np[128][None, :], b_np[None, :]], 0)  # [2,128]
    w2b2 = np.concatenate([w2row, w2row], 1).astype(NPBF16)      # [2,256]

    in_maps = []
    for c in range(N_CORES):
        lo_s, hi_s = c * spc, min((c + 1) * spc, S_sub)
        gath = np.full((spc, PAD), E, np.int64)
        if hi_s > lo_s:
            gath[:hi_s - lo_s] = em["gath"][lo_s:hi_s]
        g = ea_ext[:, gath.reshape(-1), :]              # [2, spc*PAD, 128]
        g = g.reshape(2, nsg, GRP, PAD, SUBS, PAD, 128).transpose(1, 4, 5, 2, 3, 0, 6)
        ep = np.ascontiguousarray(g.reshape(nsg, 128, GRP * 1024)).astype(NPBF16)

        lo = c * npc
        xs = np.zeros((2, npc_pad, 128), np.float32)
        xs[:, :npc] = x_np[:, lo:lo + npc]
        xt = np.ascontiguousarray(
            xs.reshape(2, ntp, 256, 128).transpose(1, 3, 0, 2).reshape(ntp, 128, 512)
        ).astype(NPBF16)
        cc_pad = np.full(npc_pad, -1.0, np.float32)
        cc_pad[:npc] = cci[lo:lo + npc].astype(np.float32)
        cc = np.ascontiguousarray(cc_pad.reshape(nt, 128).T)    # [128, nt]
        d1 = np.ones((2, npc_pad), np.float32)
        d1[0, :] = 0.0
        d1[0, :npc] = dist[lo:lo + npc]

        in_maps.append({"ep": ep, "ag": aseg, "xt": xt, "cc": cc,
                        "d1": d1.astype(NPBF16), "wf": wfull, "w2": w2b2,
                        "io": iota512})

    # ---------------- build + run ----------------
    nc = _build_program(nsg, ntp, nt)
    trace = os.environ.get("BASS_KERNEL_TRACE", "0") == "1"
    if trace:
        try:
            import profhook
            profhook.install()
        except Exception:
            trace = False
    res = run_bass_kernel_spmd(nc, in_maps, core_ids=list(range(N_CORES)),
                               trace=trace)
    LAST_EXEC_NS = res.exec_time_ns

    # ---------------- host postprocessing ----------------
    pool_sum = np.zeros((2, 128, 512), np.float64)
    for c in range(N_CORES):
        pool_sum += res.results[c]["pool"]
    pool_sum = pool_sum.astype(np.float32)
    ncnt = np.bincount(cci, minlength=N_CENTROIDS).astype(np.float32)
    node_avg = pool_sum.transpose(0, 2, 1) / np.clip(ncnt, 1.0, None)[None, :, None]

    # coarse edge attrs: gather real sub-group rows, reduce per slot
    U = em["U"]
    chunks = []
    for c in range(N_CORES):
        lo_s, hi_s = c * spc, min((c + 1) * spc, S_sub)
        if hi_s > lo_s:
            st = res.results[c]["stage"].astype(np.float32)
            st = st.reshape(nsg, 128, GRP, 256).transpose(0, 2, 1, 3).reshape(spc, 256)
            chunks.append(st[:hi_s - lo_s])
    out_attr = np.zeros((2, E_COARSE + 1, 128), np.float32)
    if U > 0:
        rows = np.concatenate(chunks, 0)                         # [S_sub, 256]
        esums = np.add.reduceat(rows, em["sub_lo"][:U], axis=0)  # [U, 256]
        emeans = esums / np.clip(em["counts"].astype(np.float32), 1.0, None)[:, None]
        out_attr[0, :U] = emeans[:, :128]
        out_attr[1, :U] = emeans[:, 128:]

    uniq = em["uniq"]
    uniq_pad = np.full(E_COARSE + 1, SENTINEL, np.int64)
    uniq_pad[:min(len(uniq), E_COARSE + 1)] = uniq[:E_COARSE + 1]
    idx_dt = np.asarray(cci_in).dtype
    if idx_dt.kind not in "iu":
        idx_dt = np.dtype(np.int32)
    ce = np.where(uniq_pad < SENTINEL,
                  np.stack([uniq_pad // N_CENTROIDS, uniq_pad % N_CENTROIDS]),
                  -1).astype(idx_dt)

    return (node_avg, out_attr, ce, np.asarray(cci_in), np.asarray(distances))


# revision 16
# speedup vs baseline: 1.0940x; 1.0813x over previous
"""Trainium2 Bass kernel for the GNN coarsening layer (nn_Coarse_layer).

Pipeline (B=2 batches, N=100k nodes, E=800k edges, H=128, C=512 centroids):
  1. fine = relu(concat([x, dist]) @ W + b)                      [B,N,H]
  2. node_avg = segment_mean(fine, cci, C)                       [B,C,H]
  3. coarse edges: group edges by sorted centroid-pair key, drop
     intra-centroid edges, unique keys -> segment_mean(edge_attr) [B,Ec+1,H]

Strategy: the host does the cheap integer index work — pair keys, unique,
argsort — and pads each coarse-edge group to a multiple of PAD=4 edges
(null edges point at an appended all-zero row).  With that layout the big
segment-sum becomes a constant-stationary matmul: fixed shifted aggregation
matrices A_j^T[128, 128] (A_j[s, e] = 1 iff s == j*32 + e//4) reduce 128
streamed edge rows into 32 partial-group rows each, all four subtiles
accumulating into one PSUM tile, with both batches side by side in the
moving operand.  The tensor engine streams with zero per-tile setup work,
PSUM accumulates in fp32, and the host divides by group counts and sums the
split sub-groups (np.add.reduceat) while assembling the output.  The tiny
MLP + 512-way node pooling run on-device as bf16 matmuls with a per-tile
iota/is_equal one-hot for the pooling reduction.  DMA traffic is batched:
GRP=4 superbins ride one 1MB load and one staging store.
"""

import os
import sys

for _p in ("/opt/trn_rl_repo", "/root/.axon_site/_ro/trn_rl_repo"):
    if os.path.isdir(_p) and _p not in sys.path:
        sys.path.append(_p)

import numpy as np
import ml_dtypes

import concourse.bacc as bacc
import concourse.tile as tile
import concourse.mybir as mybir
from concourse.bass_utils import run_bass_kernel_spmd

F32 = mybir.dt.float32
BF16 = mybir.dt.bfloat16
NPBF16 = ml_dtypes.bfloat16

N_CENTROIDS = 512
E_COARSE = N_CENTROIDS * (N_CENTROIDS - 1) // 2
SENTINEL = N_CENTROIDS * N_CENTROIDS
N_CORES = 8

PAD = 4                  # edges per sub-group (aggregation matrix row width)
SUBS = 128 // PAD        # sub-groups produced per matmul (32)
GRP = 4                  # superbins (128 sub-groups each) per DMA transfer
STAGE_F32 = False        # staging dtype; bf16 halves the store traffic

LAST_EXEC_NS = None      # filled when BASS_KERNEL_TRACE=1


def _edge_prep(edge_index, cci, E):
    """Sort edges by coarse-pair slot; pad each slot to a multiple of PAD."""
    sv = cci[edge_index[0]]
    ev = cci[edge_index[1]]
    vmin = np.minimum(sv, ev)
    vmax = np.maximum(sv, ev)
    keys = np.where(sv != ev, vmin * N_CENTROIDS + vmax, SENTINEL)
    uniq, inv = np.unique(keys, return_inverse=True)
    U = int(np.searchsorted(uniq, SENTINEL))        # valid (non-sentinel) slots
    order = np.argsort(inv, kind="stable")
    nval = int(np.searchsorted(inv[order], U))      # edges in valid slots
    order = order[:nval]
    counts = np.bincount(inv[order], minlength=max(U, 1))[:U]
    cum = np.zeros(U + 1, np.int64)
    cum[1:] = np.cumsum(counts)

    subcnt = (counts + PAD - 1) // PAD
    sub_lo = np.zeros(U + 1, np.int64)
    sub_lo[1:] = np.cumsum(subcnt)
    S_sub = int(sub_lo[-1])
    slot_of_sub = np.repeat(np.arange(U, dtype=np.int64), subcnt)
    within = np.arange(S_sub, dtype=np.int64) - sub_lo[slot_of_sub]
    epos = cum[slot_of_sub][:, None] + within[:, None] * PAD + np.arange(PAD)[None, :]
    ok = epos < cum[slot_of_sub + 1][:, None]
    epos_c = np.minimum(epos, max(nval - 1, 0))
    gath = np.where(ok, order[epos_c], E)           # E == appended zero row
    return dict(uniq=uniq, U=U, counts=counts, sub_lo=sub_lo, S_sub=S_sub,
                gath=gath)


def _build_program(nsg, ntp, nt):
    """One SPMD program; all per-core variation comes in through inputs."""
    nc = bacc.Bacc("TRN2", target_bir_lowering=False, debug=False,
                   enable_asserts=False, num_devices=N_CORES)
    stage_dt = F32 if STAGE_F32 else BF16
    ep_in = nc.dram_tensor("ep", [nsg, 128, GRP * 1024], BF16, kind="ExternalInput").ap()
    ag_in = nc.dram_tensor("ag", [128, PAD * 128], BF16, kind="ExternalInput").ap()
    xt_in = nc.dram_tensor("xt", [ntp, 128, 512], BF16, kind="ExternalInput").ap()
    cc_in = nc.dram_tensor("cc", [128, nt], F32, kind="ExternalInput").ap()
    d1_in = nc.dram_tensor("d1", [2, nt * 128], BF16, kind="ExternalInput").ap()
    w_in = nc.dram_tensor("wf", [128, 128], BF16, kind="ExternalInput").ap()
    w2_in = nc.dram_tensor("w2", [2, 256], BF16, kind="ExternalInput").ap()
    io_in = nc.dram_tensor("io", [128, 512], F32, kind="ExternalInput").ap()
    st_out = nc.dram_tensor("stage", [nsg, 128, GRP * 256], stage_dt,
                            kind="ExternalOutput").ap()
    pl_out = nc.dram_tensor("pool", [2, 128, 512], F32, kind="ExternalOutput").ap()

    relu = mybir.ActivationFunctionType.Relu
    iseq = mybir.AluOpType.is_equal

    with tile.TileContext(nc) as tc:
        with tc.tile_pool(name="consts", bufs=1) as consts, \
             tc.tile_pool(name="ep", bufs=6) as ep_pool, \
             tc.tile_pool(name="stg", bufs=6) as stg_pool, \
             tc.tile_pool(name="xp", bufs=6) as x_pool, \
             tc.tile_pool(name="fine", bufs=4) as fine_pool, \
             tc.tile_pool(name="ohn", bufs=4) as ohn_pool, \
             tc.tile_pool(name="pout", bufs=1) as pout_pool, \
             tc.tile_pool(name="eps", bufs=5, space="PSUM") as eps_pool, \
             tc.tile_pool(name="fps", bufs=1, space="PSUM") as fps_pool, \
             tc.tile_pool(name="pps", bufs=1, space="PSUM") as pps_pool:

            agt = consts.tile([128, PAD * 128], BF16, tag="agt")
            nc.sync.dma_start(agt[:], ag_in[:])
            w128 = consts.tile([128, 128], BF16, tag="w128")
            nc.sync.dma_start(w128[:], w_in[:])
            w2b2 = consts.tile([2, 256], BF16, tag="w2b2")
            nc.sync.dma_start(w2b2[:], w2_in[:])
            iot = consts.tile([128, 512], F32, tag="iot")
            nc.sync.dma_start(iot[:], io_in[:])
            cct = consts.tile([128, nt], F32, tag="cct")
            nc.sync.dma_start(cct[:], cc_in[:])
            d1t = consts.tile([2, nt * 128], BF16, tag="d1t")
            nc.sync.dma_start(d1t[:], d1_in[:])

            pool_ps = [pps_pool.tile([128, 512], F32, tag=f"pp{b}", name=f"pool_ps{b}")
                       for b in range(2)]

            def edge_group(grp):
                ept = ep_pool.tile([128, GRP * 1024], BF16, tag="ept",
                                   name=f"ept{grp}")
                nc.sync.dma_start(ept[:], ep_in[grp])
                stg = stg_pool.tile([128, GRP * 256], stage_dt, tag="stg",
                                    name=f"stg{grp}")
                for g in range(GRP):
                    eps = eps_pool.tile([128, 256], F32, tag="eps",
                                        name=f"eps{grp}_{g}")
                    for j in range(PAD):
                        nc.tensor.matmul(
                            eps[:], agt[:, j * 128:(j + 1) * 128],
                            ept[:, g * 1024 + j * 256: g * 1024 + (j + 1) * 256],
                            start=(j == 0), stop=(j == PAD - 1))
                    dst = stg[:, g * 256:(g + 1) * 256]
                    if g % 2 == 0:
                        nc.vector.tensor_copy(dst, eps[:])
                    else:
                        nc.scalar.copy(dst, eps[:])
                nc.gpsimd.dma_start(st_out[grp], stg[:])

            def node_pair(tp):
                xtt = x_pool.tile([128, 512], BF16, tag="xt", name=f"xt{tp}")
                nc.sync.dma_start(xtt[:], xt_in[tp])
                for half in range(2):
                    t = tp * 2 + half
                    ohn = ohn_pool.tile([128, 512], BF16, tag="ohn", name=f"ohn{t}")
                    nc.vector.tensor_scalar(ohn[:], iot[:], cct[:, t:t + 1], None, iseq)
                    fps = fps_pool.tile([128, 256], F32, tag="fps", name=f"fps{t}")
                    # d-term + bias first (writes all elements), then x@W halves
                    nc.tensor.matmul(fps[:], d1t[:, t * 128:(t + 1) * 128], w2b2[:],
                                     start=True, stop=False)
                    for bch in range(2):
                        nc.tensor.matmul(
                            fps[:, bch * 128:(bch + 1) * 128],
                            xtt[:, bch * 256 + half * 128: bch * 256 + (half + 1) * 128],
                            w128[:], start=False, stop=True)
                    fsb = fine_pool.tile([128, 256], BF16, tag="fsb", name=f"fsb{t}")
                    nc.scalar.activation(fsb[:], fps[:], relu)
                    for bch in range(2):
                        nc.tensor.matmul(pool_ps[bch][:],
                                         fsb[:, bch * 128:(bch + 1) * 128], ohn[:],
                                         start=(t == 0), stop=(t == nt - 1))

            for i in range(max(nsg, ntp)):
                if i < nsg:
                    edge_group(i)
                if i < ntp:
                    node_pair(i)

            for bch in range(2):
                po = pout_pool.tile([128, 512], F32, tag=f"po{bch}", name=f"po{bch}")
                nc.vector.tensor_copy(po[:], pool_ps[bch][:])
                nc.sync.dma_start(pl_out[bch], po[:])

    nc.compile()
    return nc


def kernel(x, edge_index, edge_attr, scale=None, closest_centroid_indices=None,
           distances=None, W=None, b=None, **_unused):
    global LAST_EXEC_NS
    x_np = np.asarray(x, dtype=np.float32)
    ei = np.asarray(edge_index)
    ea = np.asarray(edge_attr, dtype=np.float32)
    cci_in = closest_centroid_indices
    cci = np.asarray(cci_in).astype(np.int64)
    dist = np.asarray(distances, dtype=np.float32)
    W_np = np.asarray(W, dtype=np.float32)
    b_np = np.asarray(b, dtype=np.float32)

    B, N, H = x_np.shape
    E = ei.shape[1]
    assert H == 128 and B == 2 and N % N_CORES == 0

    # ---------------- host index prep ----------------
    em = _edge_prep(ei.astype(np.int64), cci, E)
    S_sub = em["S_sub"]
    per_grp = GRP * 128                                 # sub-groups per DMA group
    nsg = max((S_sub + N_CORES * per_grp - 1) // (N_CORES * per_grp), 1)
    spc = nsg * per_grp                                 # sub-groups per core
    npc = N // N_CORES
    nt = (npc + 127) // 128
    if nt % 2:
        nt += 1
    ntp = nt // 2
    npc_pad = nt * 128

    ea_ext = np.concatenate([ea, np.zeros((2, 1, H), np.float32)], 1)

    iota512 = np.broadcast_to(np.arange(512, dtype=np.float32), (128, 512)).copy()
    # A_j[s, e] = 1 iff s == j*32 + e//PAD; stored as A_j^T [e, s]
    e_over = np.arange(128)[:, None] // PAD
    aseg = np.concatenate([(e_over + j * SUBS == np.arange(128)[None, :])
                           for j in range(PAD)], 1).astype(NPBF16)  # [128,PAD*128]
    wfull = W_np[:128].astype(NPBF16)                            # [128,128]
    w2row = np.concatenate([W_np[128][None, :], b_np[None, :]], 0)  # [2,128]
    w2b2 = np.concatenate([w2row, w2row], 1).astype(NPBF16)      # [2,256]

    in_maps = []
    for c in range(N_CORES):
        lo_s, hi_s = c * spc, min((c + 1) * spc, S_sub)
        gath = np.full((spc, PAD), E, np.int64)
        if hi_s > lo_s:
            gath[:hi_s - lo_s] = em["gath"][lo_s:hi_s]
        g = ea_ext[:, gath.reshape(-1), :]              # [2, spc*PAD, 128]
        g = g.reshape(2, nsg, GRP, PAD, SUBS, PAD, 128).transpose(1, 4, 5, 2, 3, 0, 6)
        ep = np.ascontiguousarray(g.reshape(nsg, 128, GRP * 1024)).astype(NPBF16)

        lo = c * npc
        xs = np.zeros((2, npc_pad, 128), np.float32)
        xs[:, :npc] = x_np[:, lo:lo + npc]
        xt = np.ascontiguousarray(
            xs.reshape(2, ntp, 256, 128).transpose(1, 3, 0, 2).reshape(ntp, 128, 512)
        ).astype(NPBF16)
        cc_pad = np.full(npc_pad, -1.0, np.float32)
        cc_pad[:npc] = cci[lo:lo + npc].astype(np.float32)
        cc = np.ascontiguousarray(cc_pad.reshape(nt, 128).T)    # [128, nt]
        d1 = np.ones((2, npc_pad), np.float32)
        d1[0, :] = 0.0
        d1[0, :npc] = dist[lo:lo + npc]

        in_maps.append({"ep": ep, "ag": aseg, "xt": xt, "cc": cc,
                        "d1": d1.astype(NPBF16), "wf": wfull, "w2": w2b2,
                        "io": iota512})

    # ---------------- build + run ----------------
    nc = _build_program(nsg, ntp, nt)
    trace = os.environ.get("BASS_KERNEL_TRACE", "0") == "1"
    if trace:
        try:
            import profhook
            profhook.install()
        except Exception:
            trace = False
    res = run_bass_kernel_spmd(nc, in_maps, core_ids=list(range(N_CORES)),
                               trace=trace)
    LAST_EXEC_NS = res.exec_time_ns

    # ---------------- host postprocessing ----------------
    pool_sum = np.zeros((2, 128, 512), np.float64)
    for c in range(N_CORES):
        pool_sum += res.results[c]["pool"]
    pool_sum = pool_sum.astype(np.float32)
    ncnt = np.bincount(cci, minlength=N_CENTROIDS).astype(np.float32)
    node_avg = pool_sum.transpose(0, 2, 1) / np.clip(ncnt, 1.0, None)[None, :, None]

    # coarse edge attrs: gather real sub-group rows, reduce per slot
    U = em["U"]
    chunks = []
    for c in range(N_CORES):
        lo_s, hi_s = c * spc, min((c + 1) * spc, S_sub)
        if hi_s > lo_s:
            st = res.results[c]["stage"].astype(np.float32)
            st = st.reshape(nsg, 128, GRP, 256).transpose(0, 2, 1, 3).reshape(spc, 256)
            chunks.append(st[:hi_s - lo_s])
    out_attr = np.zeros((2, E_COARSE + 1, 128), np.float32)
    if U > 0:
        rows = np.concatenate(chunks, 0)                         # [S_sub, 256]
        esums = np.add.reduceat(rows, em["sub_lo"][:U], axis=0)  # [U, 256]
        emeans = esums / np.clip(em["counts"].astype(np.float32), 1.0, None)[:, None]
        out_attr[0, :U] = emeans[:, :128]
        out_attr[1, :U] = emeans[:, 128:]

    uniq = em["uniq"]
    uniq_pad = np.full(E_COARSE + 1, SENTINEL, np.int64)
    uniq_pad[:min(len(uniq), E_COARSE + 1)] = uniq[:E_COARSE + 1]
    idx_dt = np.asarray(cci_in).dtype
    if idx_dt.kind not in "iu":
        idx_dt = np.dtype(np.int32)
    ce = np.where(uniq_pad < SENTINEL,
                  np.stack([uniq_pad // N_CENTROIDS, uniq_pad % N_CENTROIDS]),
                  -1).astype(idx_dt)

    return (node_avg, out_attr, ce, np.asarray(cci_in), np.asarray(distances))
